# revision 2
# baseline (speedup 1.0000x reference)
"""DensityFrequencyFusion kernel for 8 Trainium2 NeuronCores.

Strategy: data-parallel over the batch dimension (B=8 -> 1 element per
NeuronCore), per the batch-independence of every op in the network. The
per-element forward pass is compiled once with jax.pmap and executed on all
8 cores simultaneously; outputs are gathered back to the full [8,256,40,40]
array on the host.

Self-contained: all shapes/constants hardcoded; no sibling imports.
"""
import numpy as np
import jax
import jax.numpy as jnp
from functools import partial

B = 8; C_IN = 256; HIDC = 128; D = 384; NH = 4; HD = D // NH; AG = 7; A = AG * AG
H5 = W5 = 20; H4 = W4 = 40; H3 = W3 = 80; OUT = 256
LN_EPS = 1e-5
_S = 0.7071067811865476  # 1/sqrt(2), db1 (Haar)
_LO = np.array([_S, _S], np.float32); _HI = np.array([_S, -_S], np.float32)
DEC = jnp.asarray(np.stack([np.outer(_LO, _LO), np.outer(_HI, _LO),
                            np.outer(_LO, _HI), np.outer(_HI, _HI)]))
REC = DEC


def conv2d(x, w, stride=1, pad=0, groups=1, bias=None):
    y = jax.lax.conv_general_dilated(x, w, (stride, stride), [(pad, pad), (pad, pad)],
                                     dimension_numbers=('NCHW', 'OIHW', 'NCHW'),
                                     feature_group_count=groups)
    return y if bias is None else y + bias[None, :, None, None]


def conv_bn_silu(x, w, g, b, stride=1, pad=0):
    y = conv2d(x, w, stride, pad)
    y = y * g[None, :, None, None] + b[None, :, None, None]
    return y * jax.nn.sigmoid(y)


def adaptive_avg_pool2d(x, oh, ow):
    H, W = x.shape[-2], x.shape[-1]
    rows = []
    for i in range(oh):
        h0 = (i * H) // oh; h1 = -((-(i + 1) * H) // oh)
        cols = []
        for j in range(ow):
            w0 = (j * W) // ow; w1 = -((-(j + 1) * W) // ow)
            cols.append(x[..., h0:h1, w0:w1].mean(axis=(-2, -1)))
        rows.append(jnp.stack(cols, -1))
    return jnp.stack(rows, -2)


def wavelet_conv(x, freq_w, freq_scale):
    Bz, C, H, W = x.shape
    xr = x.reshape(Bz, C, H // 2, 2, W // 2, 2)
    xw = jnp.einsum('bciajd,fad->bcfij', xr, DEC)
    xf = xw.reshape(Bz, C * 4, H // 2, W // 2)
    xf = freq_scale * conv2d(xf, freq_w, pad=1, groups=C * 4)
    xw = xw + xf.reshape(Bz, C, 4, H // 2, W // 2)
    y = jnp.einsum('bcfij,fad->bciajd', xw, REC)
    return y.reshape(Bz, C, H, W)


def density_attn(x, qkv_w, proj_w, proj_b, ln_g, ln_b):
    Bz, C, H, W = x.shape; N = H * W; scale = HD ** (-0.5)
    qkv = conv2d(x, qkv_w).reshape(Bz, 3, NH, HD, N)
    q = qkv[:, 0].transpose(0, 1, 3, 2)
    k = qkv[:, 1].transpose(0, 1, 3, 2)
    v = qkv[:, 2].transpose(0, 1, 3, 2)
    agent = adaptive_avg_pool2d(x, AG, AG).reshape(Bz, C, A).transpose(0, 2, 1)
    agent = agent.reshape(Bz, A, NH, HD).transpose(0, 2, 1, 3)
    qa = jax.nn.softmax(jnp.einsum('bhnd,bhad->bhna', q * scale, agent), -1)
    qc = jnp.einsum('bhna,bhad->bhnd', qa, agent)
    attn = jax.nn.softmax(jnp.einsum('bhnd,bhmd->bhnm', qc * scale, k), -1)
    xo = jnp.einsum('bhnm,bhmd->bhnd', attn, v)
    xo = xo.transpose(0, 2, 1, 3).reshape(Bz, N, C).transpose(0, 2, 1).reshape(Bz, C, H, W)
    xo = conv2d(xo, proj_w, bias=proj_b).transpose(0, 2, 3, 1)
    mu = xo.mean(-1, keepdims=True)
    var = ((xo - mu) ** 2).mean(-1, keepdims=True)
    xo = (xo - mu) * jax.lax.rsqrt(var + LN_EPS) * ln_g + ln_b
    return xo.transpose(0, 3, 1, 2)


def stat_mod(x, split_w, split_b, dw_w, dw_b, pw_w, pw_b, alpha, beta, proj_w, proj_b):
    Bz, C, H, W = x.shape; ds = 8
    s = conv2d(x, split_w, bias=split_b)
    y, xm = s[:, :C], s[:, C:]
    n = H * W
    mu = xm.mean((-2, -1), keepdims=True)
    xv = ((xm - mu) ** 2).sum((-2, -1), keepdims=True) / (n - 1)
    xs = xm.reshape(Bz, C, H // ds, ds, W // ds, ds).max((3, 5))
    xs = conv2d(xs, dw_w, pad=1, groups=C, bias=dw_b)
    xs = jax.nn.gelu(xs, approximate=False)
    xs = conv2d(xs, pw_w, bias=pw_b)
    xs = jnp.repeat(jnp.repeat(xs, ds, axis=-2), ds, axis=-1)
    xm = xm * (alpha * xs + beta * xv)
    return conv2d(xm + y, proj_w, bias=proj_b)


def _forward(x_p5, x_p4, x_p3, p5_w, p5_g, p5_b, p4_w, p4_g, p4_b, p3_w, p3_g, p3_b,
             freq_w, freq_scale, qkv_w, attn_proj_w, attn_proj_b, ln_g, ln_b,
             sm_split_w, sm_split_b, sm_dw_w, sm_dw_b, sm_pw_w, sm_pw_b,
             sm_alpha, sm_beta, sm_proj_w, sm_proj_b, out_w, out_g, out_b):
    p5 = conv_bn_silu(x_p5, p5_w, p5_g, p5_b)
    p5 = jax.image.resize(p5, (p5.shape[0], p5.shape[1], H4, W4), method='bilinear')
    p4 = conv_bn_silu(x_p4, p4_w, p4_g, p4_b)
    p3 = conv_bn_silu(x_p3, p3_w, p3_g, p3_b, stride=2, pad=1)
    xc = jnp.concatenate([p5, p4, p3], axis=1)
    xf = wavelet_conv(xc, freq_w, freq_scale)
    xa = density_attn(xf, qkv_w, attn_proj_w, attn_proj_b, ln_g, ln_b)
    xfu = xc + xa
    xm = stat_mod(xfu, sm_split_w, sm_split_b, sm_dw_w, sm_dw_b, sm_pw_w, sm_pw_b,
                  sm_alpha, sm_beta, sm_proj_w, sm_proj_b)
    xfu = xfu + xm
    return conv_bn_silu(xfu, out_w, out_g, out_b)


_ARG_ORDER = ['x_p5', 'x_p4', 'x_p3', 'p5_w', 'p5_g', 'p5_b', 'p4_w', 'p4_g', 'p4_b',
              'p3_w', 'p3_g', 'p3_b', 'freq_w', 'freq_scale', 'qkv_w', 'attn_proj_w',
              'attn_proj_b', 'ln_g', 'ln_b', 'sm_split_w', 'sm_split_b', 'sm_dw_w',
              'sm_dw_b', 'sm_pw_w', 'sm_pw_b', 'sm_alpha', 'sm_beta', 'sm_proj_w',
              'sm_proj_b', 'out_w', 'out_g', 'out_b']
_BATCH_ARGS = {'x_p5', 'x_p4', 'x_p3'}

# batch tensors sharded along axis 0 (1 element per core); weights broadcast
_pmapped = jax.pmap(
    _forward,
    axis_name='core',
    in_axes=tuple(0 if a in _BATCH_ARGS else None for a in _ARG_ORDER),
    out_axes=0,
    devices=jax.devices()[:8],
)

_warm = False


def kernel(**inputs):
    args = []
    for a in _ARG_ORDER:
        v = np.asarray(inputs[a])
        if a in _BATCH_ARGS:
            # [8, C, H, W] -> [8, 1, C, H, W]: pmap strips the device axis so
            # each core runs the forward pass on a [1, C, H, W] slice.
            v = v.reshape(8, 1, *v.shape[1:])
        args.append(v)
    out = _pmapped(*args)                 # [8, 1, 256, 40, 40]
    return np.asarray(out).reshape(8, OUT, H4, W4).astype(np.float32)
